# revision 77
# baseline (speedup 1.0000x reference)
"""Multi-head attention kernel for Trainium2, sharded over 8 NeuronCores.

Problem: B=2, S=2048, D=1024, H=16 heads (d_k=64), fp32 in/out, mask == all-ones.

Sharding: 2 heads per core (head/tensor parallel). Each core computes, for its
128-wide slice `sl` of the projection output dims:
    QT/KT = (W_*[sl] @ x.T)         -> [128, 4096]  (transposed layout, bf16)
    V     = x @ W_V[sl].T chunks    -> vaug tiles [krows, dims|ones]
    per (b, head): scoresT = K_h @ Q_h.T (contraction d_k=64, row-tiled pair)
    P.T = exp(scoresT / 8)          (no max-subtraction needed: |scores| < 8)
    acc_h = [V_h | 1].T @ P.T       -> psum; trailing row = softmax denom
    outT = acc * recip(denom) via fast-reciprocal + K=1 PE broadcast matmuls
    partialT = woT.T @ outT         -> partial.T [1024, 4096] bf16
Host: sums the 8 partials (fp32), transposes, reshapes.

Scheduling: scores/exp/attnV are software-pipelined (attnV one k-chunk behind
scores); independent matmul work (other-batch projections, V chunks, deferred
output projection) is pumped one unit per k-chunk to keep the PE continuously
busy while ACT computes exp.
"""
import sys

sys.path.insert(0, "/opt/trn_rl_repo")

from contextlib import ExitStack

import ml_dtypes
import numpy as np

import concourse.bass as bass
from concourse import bacc
import concourse.mybir as mybir
import concourse.tile as tile
from concourse.bass_utils import run_bass_kernel_spmd

BF16 = ml_dtypes.bfloat16
D = 1024
B = 2
S = 2048
BS = B * S            # 4096 rows
N_CORES = 8
SLW = D // N_CORES    # 128 output dims per core (2 heads x 64)
DK = 64
KC = S // 128         # 16 k-chunks per batch
QB = S // 512         # 4 q-blocks of 512 per batch
VW = 130              # vaug per-chunk width: [h0 V(64) | 1 | h1 V(64) | 1]
F32 = mybir.dt.float32
BF = mybir.dt.bfloat16

_nc_cache = {}
DEBUG_DUMPS = False  # when True, adds intermediate-dump outputs (dev only)


def _build_program():
    nc = bacc.Bacc("TRN2", target_bir_lowering=False, debug=False, num_devices=8)
    xT = nc.dram_tensor("xT", [D, BS], BF, kind="ExternalInput")
    # wq/wk/wv come pre-rearranged from the host into the SBUF layout
    # [p, c*SLW+m] = W[c*128+p, m] so the load is a contiguous-row DMA
    # (2KB segments) instead of a 256B-segment gather that stalls the queue.
    wqT = nc.dram_tensor("wqT", [128, 8 * SLW], BF, kind="ExternalInput")
    wkT = nc.dram_tensor("wkT", [128, 8 * SLW], BF, kind="ExternalInput")
    wvT = nc.dram_tensor("wvT", [128, 8 * SLW], BF, kind="ExternalInput")
    woT = nc.dram_tensor("woT", [SLW, D], BF, kind="ExternalInput")
    out = nc.dram_tensor("out", [D, BS], BF, kind="ExternalOutput")

    with tile.TileContext(nc) as tc, ExitStack() as ctx:
        _emit(ctx, tc, nc, xT, wqT, wkT, wvT, woT, out)
    nc.compile()
    return nc


def _emit(ctx, tc, nc, xT, wqT, wkT, wvT, woT, out):
    Exp = mybir.ActivationFunctionType.Exp

    consts = ctx.enter_context(tc.tile_pool(name="consts", bufs=1))
    big = ctx.enter_context(tc.tile_pool(name="big", bufs=1))
    stage = ctx.enter_context(tc.tile_pool(name="stage", bufs=2))
    small = ctx.enter_context(tc.tile_pool(name="small", bufs=2))
    pt_pool = ctx.enter_context(tc.tile_pool(name="pt", bufs=4))
    # PSUM (8 banks): scores 2 x [128,1024] = 4 banks; attnV accumulators
    # (held across each q-block's k loop) 2 banks; shared work ring (V/proj
    # psum, output-proj matmuls, reciprocal broadcasts) 2 banks.
    ps_sc = ctx.enter_context(tc.tile_pool(name="ps_sc", bufs=2, space="PSUM"))
    ps_acc = ctx.enter_context(tc.tile_pool(name="ps_acc", bufs=1, space="PSUM"))
    ps_work = ctx.enter_context(tc.tile_pool(name="ps_work", bufs=2, space="PSUM"))

    # ---- Input DMAs: wk/wq first (first projections), x in 32 column-major
    # pieces spread over 4 engine queues so the first projection block is
    # ready ~3us in; wv/wo ordered later on queues that are past their
    # critical pieces.
    w_sbs = {}
    for name, w in (("k", wkT), ("q", wqT), ("v", wvT)):
        w_sbs[name] = consts.tile([128, 8 * SLW], BF, tag=f"w{name}", name=f"w{name}")
    nc.sync.dma_start(w_sbs["k"][:], wkT[:])
    woT_sb = consts.tile([128, D], BF, tag="wo")

    xt_sb = [big.tile([128, BS], BF, tag=f"xt{c}", name=f"xt{c}") for c in range(8)]
    # batch-0 pieces: chunk -> queue; sync also carries wk so it gets fewer.
    q_of = {0: nc.sync, 3: nc.sync, 1: nc.scalar, 4: nc.scalar, 6: nc.scalar,
            2: nc.gpsimd, 5: nc.gpsimd, 7: nc.gpsimd}

    def xt_piece(c, p, eng=None, cols=None):
        lo, hi = cols or (p * 1024, (p + 1) * 1024)
        (eng or q_of[c]).dma_start(
            xt_sb[c][:, lo:hi], xT[c * 128 : (c + 1) * 128, lo:hi]
        )

    # Queue-priority order: wk/wq + batch-0 pieces lead their queues so the
    # interleaved k0/q0 projection d-loops can chase per-chunk arrivals; the
    # very first 512 columns of every chunk ship alone so block 0 starts
    # sooner. Batch-1 pieces stay OFF the scalar queue so exp is never stuck
    # behind DMA issues.
    nc.scalar.dma_start(w_sbs["q"][:], wqT[:])
    nc.gpsimd.dma_start(w_sbs["v"][:], wvT[:])
    for c in range(8):
        xt_piece(c, 0, cols=(0, 512))
    # interleave the second-half and p1 waves so k-blocks 1 and 2 both meet
    # their first-use deadlines (kc 4 and kc 8 of the first q-block).
    for c in range(8):
        xt_piece(c, 0, cols=(512, 1024))
        xt_piece(c, 1)
    nc.sync.dma_start(woT_sb[:], woT[:])
    for p in (2, 3):
        for c in range(8):
            xt_piece(c, p, eng=nc.sync if c in (0, 3, 6) else nc.gpsimd)
    # low half of woT re-based at partition 0: lets the final q-block's
    # output projection consume the head-1 half (tmp, partitions 0:64)
    # directly via split-K accumulation, skipping the tmp->outT DMA that
    # otherwise sits on the tail's critical path.
    woT_lo = consts.tile([64, D], BF, tag="wo_lo", name="wo_lo")
    nc.gpsimd.dma_start(woT_lo[:], woT_sb[64:128, :])

    # ---- Constants: vaug ones-columns (memset 1.0; V copies overwrite data
    # cols, pad cols harmlessly stay 1), K=1 broadcast rows at partitions
    # 64 and 96 (legal PE tile_position rows).
    # memsets on DVE: it's idle at startup and doesn't carry input DMAs.
    # vaug: per 128-row chunk vc, cols [vc*130 .. ] hold [h0 V | 1 | h1 V | 1];
    # the attnV stationaries are the two 65-wide halves, so both heads'
    # denominators land on psum row 64. Ones columns survive the V copies.
    vaug = consts.tile([128, KC * B * VW], BF, tag="vaug", name="vaug")
    nc.vector.memset(vaug[:], 1.0)
    ones_t = consts.tile([128, 64], BF, tag="ones_t", name="ones_t")
    nc.vector.memset(ones_t[64:65, :], 1.0)
    # reciprocal_approx_fast mis-executes on single-partition slices on HW,
    # so both denominators ride in row 64 of a full-height [128, 1024] tile
    # (head 0 in cols 0:512, head 1 in 512:1024) and the reciprocal runs over
    # all 128 partitions (same cycle count — cost is free-dim size). Unused
    # rows hold 1.0 so the op stays in-range.
    den_t = consts.tile([128, 1024], F32, tag="den_t", name="den_t")
    rec_t = consts.tile([128, 1024], F32, tag="rec_t", name="rec_t")
    nc.vector.memset(den_t[:], 1.0)

    proj = {}
    for name in ("k", "q"):
        proj[name] = big.tile([128, BS], BF, tag=f"{name}T", name=f"{name}T")

    def gen_qk_block(name, n):
        # n: global 512-col block index (0..7) over both batches.
        dst = proj[name]
        w_sb = w_sbs[name]
        ps = ps_work.tile([128, 512], F32, tag="work", name=f"ps_{name}{n}")
        for d in range(8):
            nc.tensor.matmul(
                ps[:],
                w_sb[:, bass.ts(d, SLW)],
                xt_sb[d][:, bass.ts(n, 512)],
                start=(d == 0),
                stop=(d == 7),
            )
            yield
        nc.vector.tensor_copy(dst[:, bass.ts(n, 512)], ps[:])

    def emit_qk_block(name, n):
        for _ in gen_qk_block(name, n):
            pass

    def gen_v_chunk(rc):
        # rc: global 128-row chunk (0..31). One strided copy drops both
        # heads' V columns into the vaug layout around the ones columns.
        wv_sb = w_sbs["v"]
        ps = ps_work.tile([128, 128], F32, tag="work", name=f"ps_v{rc}")
        for d in range(8):
            nc.tensor.matmul(
                ps[:],
                xt_sb[d][:, bass.ts(rc, 128)],
                wv_sb[:, bass.ts(d, SLW)],
                start=(d == 0),
                stop=(d == 7),
            )
            yield
        dst = vaug[:, rc * VW : (rc + 1) * VW].rearrange(
            "p (two m) -> p two m", two=2
        )[:, :, 0:64]
        src = ps[:].rearrange("p (two m) -> p two m", two=2)[:, :, 0:64]
        nc.vector.tensor_copy(dst, src)

    def emit_v_chunk(rc):
        for _ in gen_v_chunk(rc):
            pass

    # ---- Filler units: independent PE work pumped one whole unit per
    # k-chunk. Unit granularity matters: back-to-back short matmuls inside a
    # unit hide each other's stationary loads poorly, but splicing them
    # between long attention streams is worse — every long matmul following a
    # short stream exposes its own stationary load. ensure(key) drains a
    # unit before its consumer emits.
    filler = []  # list of (key, generator-factory)
    cur = [None, None]  # kept for drain-loop compatibility

    def drain(gen):
        for _ in gen:
            pass

    def pump():
        if filler:
            _, gf = filler.pop(0)
            drain(gf())

    def ensure(key):
        for i, (k, gf) in enumerate(filler):
            if k == key:
                filler.pop(i)
                drain(gf())
                return

    # ---- Output projection, one q-block of one batch = 8 jc units + 1 DMA.
    # In tail mode (final q-block) the psum->sbuf casts alternate DVE/ACT
    # (both otherwise idle) and the store is split so DMA overlaps the casts.
    Copy = mybir.ActivationFunctionType.Copy

    state = {}

    def push_oproj(b, qb, outT, tail=False):
        st = stage.tile([128, 8 * 512], BF, tag="st", name=f"st{b}_{qb}")

        def store(jc_lo, jc_hi, eng):
            cols = slice(b * S + qb * 512 + 0, b * S + (qb + 1) * 512)
            eng.dma_start(
                out[:, cols]
                .rearrange("(c p) m -> p c m", p=128)[:, jc_lo:jc_hi, :],
                st[:].rearrange("p (c m) -> p c m", c=8)[:, jc_lo:jc_hi, :],
            )

        def mk(jc):
            def unit():
                pm = ps_work.tile([128, 512], F32, tag="work", name=f"pm{b}_{qb}_{jc}")
                if tail:
                    # split-K: head-1 half comes straight from tmp (partitions
                    # 0:64) so the tail never waits on the tmp->outT DMA.
                    nc.tensor.matmul(
                        pm[:],
                        woT_sb[0:64, bass.ts(jc, 128)],
                        outT[0:64, bass.ts(qb, 512)],
                        start=True,
                        stop=False,
                    )
                    nc.tensor.matmul(
                        pm[:],
                        woT_lo[:, bass.ts(jc, 128)],
                        state["tmp"][:],
                        start=False,
                        stop=True,
                    )
                else:
                    nc.tensor.matmul(
                        pm[:],
                        woT_sb[:, bass.ts(jc, 128)],
                        outT[:, bass.ts(qb, 512)],
                        start=True,
                        stop=True,
                    )
                if tail and jc % 2 == 1:
                    nc.scalar.activation(st[:, bass.ts(jc, 512)], pm[:], Copy)
                else:
                    nc.vector.tensor_copy(st[:, bass.ts(jc, 512)], pm[:])
                if tail and jc % 2 == 1:
                    # quarter stores alternate queues so the final transfer
                    # is small and overlaps the remaining casts.
                    store(jc - 1, jc + 1, nc.sync if jc % 4 == 1 else nc.scalar)
                elif not tail and jc == 7:
                    store(0, 8, nc.sync)

            def g():
                unit()
                yield

            return g

        for jc in range(8):
            filler.append((("o", b, qb, jc), mk(jc)))

    # ---- All projection work flows through the filler queue; scores(kc)
    # only needs k-block kc//4, so attention starts right after k0+q0,
    # whose d-loops interleave to chase per-chunk DMA arrivals.
    import itertools

    for _ in itertools.zip_longest(gen_qk_block("k", 0), gen_qk_block("q", 0)):
        pass
    for n in range(1, QB):
        filler.append((("p", "k", n), lambda n=n: gen_qk_block("k", n)))
    for n in range(1, QB):
        filler.append((("p", "q", n), lambda n=n: gen_qk_block("q", n)))
    for n in range(QB, 2 * QB):
        for name in ("k", "q"):
            filler.append(
                (("p", name, n), lambda name=name, n=n: gen_qk_block(name, n))
            )
    for rc in range(KC, 2 * KC):
        filler.append((("v", rc), lambda rc=rc: gen_v_chunk(rc)))

    # ---- Attention, software-pipelined: per k-chunk emit scores(kc),
    # exp(kc), one filler unit, attnV(kc-1).
    def emit_attnv(b, qb, kc, pt, acc0, acc1):
        vc = b * KC + kc
        nc.tensor.matmul(
            acc0[:],
            vaug[:, vc * VW : vc * VW + 65],
            pt[:, 0:512],
            start=(kc == 0),
            stop=(kc == KC - 1),
        )
        nc.tensor.matmul(
            acc1[:],
            vaug[:, vc * VW + 65 : (vc + 1) * VW],
            pt[:, 512:1024],
            start=(kc == 0),
            stop=(kc == KC - 1),
        )

    qT, kT = proj["q"], proj["k"]
    last = (B - 1, QB - 1)
    for b in range(B):
        outT = big.tile([128, S], BF, tag=f"outT{b}", name=f"outT{b}")
        for qb in range(QB):
            # deadline: the q projection this q-block's scores read must
            # exist before the reads are emitted (Tile deps follow emission
            # order); k-blocks are ensured per k-chunk inside the loop.
            ensure(("p", "q", b * QB + qb))
            q0 = b * S + qb * 512
            acc0 = ps_acc.tile([65, 512], F32, tag="acc0", name=f"acc0_{b}_{qb}")
            acc1 = ps_acc.tile([65, 512], F32, tag="acc1", name=f"acc1_{b}_{qb}")
            pts = [None] * KC
            for kc in range(KC):
                ensure(("p", "k", b * QB + kc // 4))
                k0 = b * S + kc * 128
                inline_v = b == 0 and qb == 0 and 2 * kc < KC
                # endgame: hold filler units back so they run during the
                # final normalize chain, keeping the PE warm (p-state) right
                # before the tail's output projection.
                hold = (b, qb) == last or (
                    b == B - 1 and qb == QB - 2 and kc >= 4
                )
                sc = ps_sc.tile([128, 1024], F32, tag="sc", name=f"sc{b}_{qb}_{kc}")
                for h in range(2):
                    nc.tensor.matmul(
                        sc[:, bass.ts(h, 512)],
                        kT[h * 64 : (h + 1) * 64, k0 : k0 + 128],
                        qT[h * 64 : (h + 1) * 64, q0 : q0 + 512],
                        start=True,
                        stop=True,
                    )
                pt = pt_pool.tile([128, 1024], BF, tag="pt", name=f"pt{b}_{qb}_{kc}")
                nc.scalar.activation(pt[:], sc[:], Exp, scale=0.125)
                pts[kc] = pt
                if DEBUG_DUMPS and b == 0 and qb == 0 and kc == 0:
                    dbgp = nc.dram_tensor("dbg_pt", [128, 1024], BF, kind="ExternalOutput")
                    nc.sync.dma_start(dbgp[:], pt[:])
                if inline_v:
                    # batch-0 V chunks are deadline-critical: chunk kc must
                    # exist before attnV(kc); 2 per k-chunk stays ahead.
                    emit_v_chunk(2 * kc)
                    emit_v_chunk(2 * kc + 1)
                elif not hold:
                    pump()
                # attnV runs two k-chunks behind scores: exp(kc) gets ~2
                # k-chunks of PE work as lead time, so attnV never
                # head-of-line blocks on ACT, even across q-block boundaries
                # where ACT carries a 3-exp backlog.
                if kc >= 2:
                    ensure(("v", b * KC + kc - 2))
                    emit_attnv(b, qb, kc - 2, pts[kc - 2], acc0, acc1)
            for j in (KC - 2, KC - 1):
                ensure(("v", b * KC + j))
                emit_attnv(b, qb, j, pts[j], acc0, acc1)

            # ---- normalize ----
            # custom-DVE ops can't read PSUM on hardware: hop the denominator
            # rows through SBUF before the fast reciprocal. The acc data rows
            # also hop to SBUF immediately — this frees the acc PSUM banks for
            # the next q-block and satisfies the TensorTensor one-PSUM rule
            # (the muls then read sbuf-acc x psum-rep).
            rec_bf = small.tile([65, 1024], BF, tag="rec_bf", name=f"recb{b}_{qb}")
            a0_sb = small.tile([64, 512], F32, tag="a0sb", name=f"a0s{b}_{qb}")
            a1_sb = small.tile([64, 512], F32, tag="a1sb", name=f"a1s{b}_{qb}")
            if (b, qb) == last:
                # tail: den rows hop via the otherwise-idle ACT engine so the
                # reciprocal chain and the acc copies run in parallel, and the
                # reciprocal/cast split per head so bcast0 starts sooner.
                nc.scalar.activation(den_t[64:65, 0:512], acc0[64:65, :], Copy)
                nc.scalar.activation(den_t[64:65, 512:1024], acc1[64:65, :], Copy)
                nc.vector.reciprocal_approx_fast(rec_t[:, 0:512], den_t[:, 0:512])
                nc.vector.tensor_copy(rec_bf[64:65, 0:512], rec_t[64:65, 0:512])
                nc.vector.tensor_copy(a0_sb[:], acc0[0:64, :])
                nc.vector.reciprocal_approx_fast(rec_t[:, 512:1024], den_t[:, 512:1024])
                nc.vector.tensor_copy(rec_bf[64:65, 512:1024], rec_t[64:65, 512:1024])
                nc.vector.tensor_copy(a1_sb[:], acc1[0:64, :])
            else:
                nc.vector.tensor_copy(den_t[64:65, 0:512], acc0[64:65, :])
                nc.vector.tensor_copy(den_t[64:65, 512:1024], acc1[64:65, :])
                nc.vector.reciprocal_approx_fast(rec_t[:], den_t[:])
                nc.vector.tensor_copy(rec_bf[64:65, :], rec_t[64:65, :])
                nc.vector.tensor_copy(a0_sb[:], acc0[0:64, :])
                nc.vector.tensor_copy(a1_sb[:], acc1[0:64, :])

            def norm_post():
                # NOTE: gpsimd.partition_broadcast always reads partition 0
                # on hardware (AP partition offset ignored), so the recip
                # rows at partition 64 must broadcast via PE matmuls.
                rep0 = ps_work.tile([64, 512], F32, tag="work", name=f"rep0_{b}_{qb}")
                nc.tensor.matmul(
                    rep0[:], ones_t[64:65, :], rec_bf[64:65, 0:512],
                    start=True, stop=True, tile_position=(64, 0),
                )
                rep1 = ps_work.tile([64, 512], F32, tag="work", name=f"rep1_{b}_{qb}")
                nc.tensor.matmul(
                    rep1[:], ones_t[64:65, :], rec_bf[64:65, 512:1024],
                    start=True, stop=True, tile_position=(64, 0),
                )
                nc.vector.tensor_mul(
                    outT[0:64, bass.ts(qb, 512)], a0_sb[:], rep0[:]
                )
                tmp = small.tile([64, 512], BF, tag="tmp", name=f"tmp{b}_{qb}")
                nc.vector.tensor_mul(tmp[:], a1_sb[:], rep1[:])
                state["tmp"] = tmp
                if (b, qb) != last:
                    nc.sync.dma_start(outT[64:128, bass.ts(qb, 512)], tmp[:])
                if DEBUG_DUMPS and b == 0 and qb == 0:
                    for dn, dt_ in (
                        ("rec", rec_t[64:65, 0:512]),
                        ("rec1", rec_t[64:65, 512:1024]),
                        ("rep0", a0_sb[:]),
                        ("rep1", a1_sb[:]),
                        ("vaug0", vaug[:, 0:65]),
                        ("vaug1", vaug[:, 65:VW]),
                    ):
                        dbgt = nc.dram_tensor(
                            f"dbg_{dn}", list(dt_.shape), dt_.dtype, kind="ExternalOutput"
                        )
                        nc.sync.dma_start(dbgt[:], dt_)

            if DEBUG_DUMPS and qb == QB - 1:
                dbgo = nc.dram_tensor(f"dbg_outT{b}", [128, S], BF, kind="ExternalOutput")
                nc.sync.dma_start(dbgo[:], outT[:])
            if (b, qb) == last:
                # tail: held filler units execute on the PE while the DVE/ACT
                # normalize chain runs, then the split-K output projection.
                while filler or cur[1] is not None:
                    pump()
                norm_post()
                push_oproj(b, qb, outT, tail=True)
                while filler or cur[1] is not None:
                    pump()
            else:
                norm_post()
                push_oproj(b, qb, outT)


def kernel(x, mask, W_Q, W_K, W_V, W_O, _trace=False):
    # mask is all-ones for this problem; the reference `where(mask==0, -inf)` is a
    # no-op, so it is not shipped to the device.
    x = np.ascontiguousarray(np.asarray(x), dtype=np.float32)
    xT_bf = np.ascontiguousarray(np.asarray(x).reshape(BS, D).T).astype(BF16)

    if "nc" not in _nc_cache:
        _nc_cache["nc"] = _build_program()
    nc = _nc_cache["nc"]

    def prep_w(W, c):
        # device SBUF layout [p, cc*SLW + m] = W[sl].T[cc*128 + p, m]
        sl = slice(c * SLW, (c + 1) * SLW)
        wT = np.asarray(W)[sl, :].T.reshape(8, 128, SLW)  # [cc, p, m]
        return np.ascontiguousarray(wT.transpose(1, 0, 2).reshape(128, 8 * SLW)).astype(
            BF16
        )

    in_maps = []
    for c in range(N_CORES):
        sl = slice(c * SLW, (c + 1) * SLW)
        in_maps.append(
            {
                "xT": xT_bf,
                "wqT": prep_w(W_Q, c),
                "wkT": prep_w(W_K, c),
                "wvT": prep_w(W_V, c),
                "woT": np.ascontiguousarray(np.asarray(W_O)[:, sl].T).astype(BF16),
            }
        )

    res = run_bass_kernel_spmd(nc, in_maps, core_ids=list(range(N_CORES)), trace=_trace)
    _nc_cache["last_result"] = res

    total = np.zeros((D, BS), dtype=np.float32)
    for c in range(N_CORES):
        total += res.results[c]["out"].astype(np.float32)
    return np.ascontiguousarray(total.T).reshape(B, S, D)


# revision 78
# speedup vs baseline: 1.1710x; 1.1710x over previous
"""Multi-head attention kernel for Trainium2, sharded over 8 NeuronCores.

Problem: B=2, S=2048, D=1024, H=16 heads (d_k=64), fp32 in/out, mask == all-ones.

Sharding: 2 heads per core (head/tensor parallel). Each core computes, for its
128-wide slice `sl` of the projection output dims:
    QT/KT = (W_*[sl] @ x.T)         -> [128, 4096]  (transposed layout, bf16)
    V     = x @ W_V[sl].T chunks    -> vaug tiles [krows, dims|ones]
    per (b, head): scoresT = K_h @ Q_h.T (contraction d_k=64, row-tiled pair)
    P.T = exp(scoresT / 8)          (no max-subtraction needed: |scores| < 8)
    acc_h = [V_h | 1].T @ P.T       -> psum; trailing row = softmax denom
    outT = acc * recip(denom) via fast-reciprocal + K=1 PE broadcast matmuls
    partialT = woT.T @ outT         -> partial.T [1024, 4096] bf16
Host: sums the 8 partials (fp32), transposes, reshapes.

Scheduling: scores/exp/attnV are software-pipelined (attnV one k-chunk behind
scores); independent matmul work (other-batch projections, V chunks, deferred
output projection) is pumped one unit per k-chunk to keep the PE continuously
busy while ACT computes exp.
"""
import sys

sys.path.insert(0, "/opt/trn_rl_repo")

from contextlib import ExitStack

import ml_dtypes
import numpy as np

import concourse.bass as bass
from concourse import bacc
import concourse.mybir as mybir
import concourse.tile as tile
from concourse.bass_utils import run_bass_kernel_spmd

BF16 = ml_dtypes.bfloat16
D = 1024
B = 2
S = 2048
BS = B * S            # 4096 rows
N_CORES = 8
SLW = D // N_CORES    # 128 output dims per core (2 heads x 64)
DK = 64
KC = S // 128         # 16 k-chunks per batch
QB = S // 512         # 4 q-blocks of 512 per batch
VW = 130              # vaug per-chunk width: [h0 V(64) | 1 | h1 V(64) | 1]
F32 = mybir.dt.float32
BF = mybir.dt.bfloat16

_nc_cache = {}
DEBUG_DUMPS = False  # when True, adds intermediate-dump outputs (dev only)


def _build_program():
    nc = bacc.Bacc("TRN2", target_bir_lowering=False, debug=False, num_devices=8)
    xT = nc.dram_tensor("xT", [D, BS], BF, kind="ExternalInput")
    # wq/wk/wv come pre-rearranged from the host into the SBUF layout
    # [p, c*SLW+m] = W[c*128+p, m] so the load is a contiguous-row DMA
    # (2KB segments) instead of a 256B-segment gather that stalls the queue.
    wqT = nc.dram_tensor("wqT", [128, 8 * SLW], BF, kind="ExternalInput")
    wkT = nc.dram_tensor("wkT", [128, 8 * SLW], BF, kind="ExternalInput")
    wvT = nc.dram_tensor("wvT", [128, 8 * SLW], BF, kind="ExternalInput")
    woT = nc.dram_tensor("woT", [SLW, D], BF, kind="ExternalInput")
    out = nc.dram_tensor("out", [D, BS], BF, kind="ExternalOutput")

    with tile.TileContext(nc) as tc, ExitStack() as ctx:
        _emit(ctx, tc, nc, xT, wqT, wkT, wvT, woT, out)
    nc.compile()
    return nc


def _emit(ctx, tc, nc, xT, wqT, wkT, wvT, woT, out):
    Exp = mybir.ActivationFunctionType.Exp

    consts = ctx.enter_context(tc.tile_pool(name="consts", bufs=1))
    big = ctx.enter_context(tc.tile_pool(name="big", bufs=1))
    stage = ctx.enter_context(tc.tile_pool(name="stage", bufs=2))
    small = ctx.enter_context(tc.tile_pool(name="small", bufs=2))
    pt_pool = ctx.enter_context(tc.tile_pool(name="pt", bufs=4))
    # PSUM (8 banks): scores 2 x [128,1024] = 4 banks; attnV accumulators
    # (held across each q-block's k loop) 2 banks; shared work ring (V/proj
    # psum, output-proj matmuls, reciprocal broadcasts) 2 banks.
    ps_sc = ctx.enter_context(tc.tile_pool(name="ps_sc", bufs=2, space="PSUM"))
    ps_acc = ctx.enter_context(tc.tile_pool(name="ps_acc", bufs=1, space="PSUM"))
    ps_work = ctx.enter_context(tc.tile_pool(name="ps_work", bufs=2, space="PSUM"))

    # ---- Input DMAs: wk/wq first (first projections), x in 32 column-major
    # pieces spread over 4 engine queues so the first projection block is
    # ready ~3us in; wv/wo ordered later on queues that are past their
    # critical pieces.
    w_sbs = {}
    for name, w in (("k", wkT), ("q", wqT), ("v", wvT)):
        w_sbs[name] = consts.tile([128, 8 * SLW], BF, tag=f"w{name}", name=f"w{name}")
    nc.sync.dma_start(w_sbs["k"][:], wkT[:])
    woT_sb = consts.tile([128, D], BF, tag="wo")

    xt_sb = [big.tile([128, BS], BF, tag=f"xt{c}", name=f"xt{c}") for c in range(8)]
    # batch-0 pieces: chunk -> queue; sync also carries wk so it gets fewer.
    q_of = {0: nc.sync, 3: nc.sync, 1: nc.scalar, 4: nc.scalar, 6: nc.scalar,
            2: nc.gpsimd, 5: nc.gpsimd, 7: nc.gpsimd}

    def xt_piece(c, p, eng=None, cols=None):
        lo, hi = cols or (p * 1024, (p + 1) * 1024)
        (eng or q_of[c]).dma_start(
            xt_sb[c][:, lo:hi], xT[c * 128 : (c + 1) * 128, lo:hi]
        )

    # Queue-priority order: wk/wq + batch-0 pieces lead their queues so the
    # interleaved k0/q0 projection d-loops can chase per-chunk arrivals; the
    # very first 512 columns of every chunk ship alone so block 0 starts
    # sooner. Batch-1 pieces stay OFF the scalar queue so exp is never stuck
    # behind DMA issues.
    nc.scalar.dma_start(w_sbs["q"][:], wqT[:])
    nc.gpsimd.dma_start(w_sbs["v"][:], wvT[:])
    for c in range(8):
        xt_piece(c, 0, cols=(0, 512))
    for c in range(8):
        xt_piece(c, 0, cols=(512, 1024))
    for c in range(8):
        xt_piece(c, 1)
    nc.sync.dma_start(woT_sb[:], woT[:])
    for p in (2, 3):
        for c in range(8):
            xt_piece(c, p, eng=nc.sync if c in (0, 3, 6) else nc.gpsimd)
    # low half of woT re-based at partition 0: lets the final q-block's
    # output projection consume the head-1 half (tmp, partitions 0:64)
    # directly via split-K accumulation, skipping the tmp->outT DMA that
    # otherwise sits on the tail's critical path.
    woT_lo = consts.tile([64, D], BF, tag="wo_lo", name="wo_lo")
    nc.gpsimd.dma_start(woT_lo[:], woT_sb[64:128, :])

    # ---- Constants: vaug ones-columns (memset 1.0; V copies overwrite data
    # cols, pad cols harmlessly stay 1), K=1 broadcast rows at partitions
    # 64 and 96 (legal PE tile_position rows).
    # memsets on DVE: it's idle at startup and doesn't carry input DMAs.
    # vaug: per 128-row chunk vc, cols [vc*130 .. ] hold [h0 V | 1 | h1 V | 1];
    # the attnV stationaries are the two 65-wide halves, so both heads'
    # denominators land on psum row 64. Ones columns survive the V copies.
    vaug = consts.tile([128, KC * B * VW], BF, tag="vaug", name="vaug")
    nc.vector.memset(vaug[:], 1.0)
    ones_t = consts.tile([128, 64], BF, tag="ones_t", name="ones_t")
    nc.vector.memset(ones_t[64:65, :], 1.0)
    # reciprocal_approx_fast mis-executes on single-partition slices on HW,
    # so both denominators ride in row 64 of a full-height [128, 1024] tile
    # (head 0 in cols 0:512, head 1 in 512:1024) and the reciprocal runs over
    # all 128 partitions (same cycle count — cost is free-dim size). Unused
    # rows hold 1.0 so the op stays in-range.
    den_t = consts.tile([128, 1024], F32, tag="den_t", name="den_t")
    rec_t = consts.tile([128, 1024], F32, tag="rec_t", name="rec_t")
    nc.vector.memset(den_t[:], 1.0)

    proj = {}
    for name in ("k", "q"):
        proj[name] = big.tile([128, BS], BF, tag=f"{name}T", name=f"{name}T")

    def gen_qk_block(name, n):
        # n: global 512-col block index (0..7) over both batches.
        dst = proj[name]
        w_sb = w_sbs[name]
        ps = ps_work.tile([128, 512], F32, tag="work", name=f"ps_{name}{n}")
        for d in range(8):
            nc.tensor.matmul(
                ps[:],
                w_sb[:, bass.ts(d, SLW)],
                xt_sb[d][:, bass.ts(n, 512)],
                start=(d == 0),
                stop=(d == 7),
            )
            yield
        nc.vector.tensor_copy(dst[:, bass.ts(n, 512)], ps[:])

    def emit_qk_block(name, n):
        for _ in gen_qk_block(name, n):
            pass

    def gen_v_chunk(rc):
        # rc: global 128-row chunk (0..31). One strided copy drops both
        # heads' V columns into the vaug layout around the ones columns.
        wv_sb = w_sbs["v"]
        ps = ps_work.tile([128, 128], F32, tag="work", name=f"ps_v{rc}")
        for d in range(8):
            nc.tensor.matmul(
                ps[:],
                xt_sb[d][:, bass.ts(rc, 128)],
                wv_sb[:, bass.ts(d, SLW)],
                start=(d == 0),
                stop=(d == 7),
            )
            yield
        dst = vaug[:, rc * VW : (rc + 1) * VW].rearrange(
            "p (two m) -> p two m", two=2
        )[:, :, 0:64]
        src = ps[:].rearrange("p (two m) -> p two m", two=2)[:, :, 0:64]
        nc.vector.tensor_copy(dst, src)

    def emit_v_chunk(rc):
        for _ in gen_v_chunk(rc):
            pass

    # ---- Filler units: independent PE work pumped one whole unit per
    # k-chunk. Unit granularity matters: back-to-back short matmuls inside a
    # unit hide each other's stationary loads poorly, but splicing them
    # between long attention streams is worse — every long matmul following a
    # short stream exposes its own stationary load. ensure(key) drains a
    # unit before its consumer emits.
    filler = []  # list of (key, generator-factory)
    cur = [None, None]  # kept for drain-loop compatibility

    def drain(gen):
        for _ in gen:
            pass

    def pump():
        if filler:
            _, gf = filler.pop(0)
            drain(gf())

    def ensure(key):
        for i, (k, gf) in enumerate(filler):
            if k == key:
                filler.pop(i)
                drain(gf())
                return

    # ---- Output projection, one q-block of one batch = 8 jc units + 1 DMA.
    # In tail mode (final q-block) the psum->sbuf casts alternate DVE/ACT
    # (both otherwise idle) and the store is split so DMA overlaps the casts.
    Copy = mybir.ActivationFunctionType.Copy

    state = {}

    def push_oproj(b, qb, outT, tail=False):
        st = stage.tile([128, 8 * 512], BF, tag="st", name=f"st{b}_{qb}")

        def store(jc_lo, jc_hi, eng):
            cols = slice(b * S + qb * 512 + 0, b * S + (qb + 1) * 512)
            eng.dma_start(
                out[:, cols]
                .rearrange("(c p) m -> p c m", p=128)[:, jc_lo:jc_hi, :],
                st[:].rearrange("p (c m) -> p c m", c=8)[:, jc_lo:jc_hi, :],
            )

        def mk(jc):
            def unit():
                pm = ps_work.tile([128, 512], F32, tag="work", name=f"pm{b}_{qb}_{jc}")
                if tail:
                    # split-K: head-1 half comes straight from tmp (partitions
                    # 0:64) so the tail never waits on the tmp->outT DMA.
                    nc.tensor.matmul(
                        pm[:],
                        woT_sb[0:64, bass.ts(jc, 128)],
                        outT[0:64, bass.ts(qb, 512)],
                        start=True,
                        stop=False,
                    )
                    nc.tensor.matmul(
                        pm[:],
                        woT_lo[:, bass.ts(jc, 128)],
                        state["tmp"][:],
                        start=False,
                        stop=True,
                    )
                else:
                    nc.tensor.matmul(
                        pm[:],
                        woT_sb[:, bass.ts(jc, 128)],
                        outT[:, bass.ts(qb, 512)],
                        start=True,
                        stop=True,
                    )
                if tail and jc % 2 == 1:
                    nc.scalar.activation(st[:, bass.ts(jc, 512)], pm[:], Copy)
                else:
                    nc.vector.tensor_copy(st[:, bass.ts(jc, 512)], pm[:])
                if tail and jc % 2 == 1:
                    # quarter stores alternate queues so the final transfer
                    # is small and overlaps the remaining casts.
                    store(jc - 1, jc + 1, nc.sync if jc % 4 == 1 else nc.scalar)
                elif not tail and jc == 7:
                    store(0, 8, nc.sync)

            def g():
                unit()
                yield

            return g

        for jc in range(8):
            filler.append((("o", b, qb, jc), mk(jc)))

    # ---- All projection work flows through the filler queue; scores(kc)
    # only needs k-block kc//4, so attention starts right after k0+q0,
    # whose d-loops interleave to chase per-chunk DMA arrivals.
    import itertools

    for _ in itertools.zip_longest(gen_qk_block("k", 0), gen_qk_block("q", 0)):
        pass
    for n in range(1, QB):
        filler.append((("p", "k", n), lambda n=n: gen_qk_block("k", n)))
    for n in range(1, QB):
        filler.append((("p", "q", n), lambda n=n: gen_qk_block("q", n)))
    for n in range(QB, 2 * QB):
        for name in ("k", "q"):
            filler.append(
                (("p", name, n), lambda name=name, n=n: gen_qk_block(name, n))
            )
    for rc in range(KC, 2 * KC):
        filler.append((("v", rc), lambda rc=rc: gen_v_chunk(rc)))

    # ---- Attention, software-pipelined: per k-chunk emit scores(kc),
    # exp(kc), one filler unit, attnV(kc-1).
    def emit_attnv(b, qb, kc, pt, acc0, acc1):
        vc = b * KC + kc
        nc.tensor.matmul(
            acc0[:],
            vaug[:, vc * VW : vc * VW + 65],
            pt[:, 0:512],
            start=(kc == 0),
            stop=(kc == KC - 1),
        )
        nc.tensor.matmul(
            acc1[:],
            vaug[:, vc * VW + 65 : (vc + 1) * VW],
            pt[:, 512:1024],
            start=(kc == 0),
            stop=(kc == KC - 1),
        )

    qT, kT = proj["q"], proj["k"]
    last = (B - 1, QB - 1)
    for b in range(B):
        outT = big.tile([128, S], BF, tag=f"outT{b}", name=f"outT{b}")
        for qb in range(QB):
            # deadline: the q projection this q-block's scores read must
            # exist before the reads are emitted (Tile deps follow emission
            # order); k-blocks are ensured per k-chunk inside the loop.
            ensure(("p", "q", b * QB + qb))
            q0 = b * S + qb * 512
            acc0 = ps_acc.tile([65, 512], F32, tag="acc0", name=f"acc0_{b}_{qb}")
            acc1 = ps_acc.tile([65, 512], F32, tag="acc1", name=f"acc1_{b}_{qb}")
            pts = [None] * KC
            for kc in range(KC):
                ensure(("p", "k", b * QB + kc // 4))
                k0 = b * S + kc * 128
                inline_v = b == 0 and qb == 0 and 2 * kc < KC
                # endgame: hold filler units back so they run during the
                # final normalize chain, keeping the PE warm (p-state) right
                # before the tail's output projection.
                hold = (b, qb) == last or (
                    b == B - 1 and qb == QB - 2 and kc >= 4
                )
                sc = ps_sc.tile([128, 1024], F32, tag="sc", name=f"sc{b}_{qb}_{kc}")
                for h in range(2):
                    nc.tensor.matmul(
                        sc[:, bass.ts(h, 512)],
                        kT[h * 64 : (h + 1) * 64, k0 : k0 + 128],
                        qT[h * 64 : (h + 1) * 64, q0 : q0 + 512],
                        start=True,
                        stop=True,
                    )
                pt = pt_pool.tile([128, 1024], BF, tag="pt", name=f"pt{b}_{qb}_{kc}")
                nc.scalar.activation(pt[:], sc[:], Exp, scale=0.125)
                pts[kc] = pt
                if DEBUG_DUMPS and b == 0 and qb == 0 and kc == 0:
                    dbgp = nc.dram_tensor("dbg_pt", [128, 1024], BF, kind="ExternalOutput")
                    nc.sync.dma_start(dbgp[:], pt[:])
                if inline_v:
                    # batch-0 V chunks are deadline-critical: chunk kc must
                    # exist before attnV(kc); 2 per k-chunk stays ahead.
                    emit_v_chunk(2 * kc)
                    emit_v_chunk(2 * kc + 1)
                elif not hold:
                    pump()
                # attnV runs two k-chunks behind scores: exp(kc) gets ~2
                # k-chunks of PE work as lead time, so attnV never
                # head-of-line blocks on ACT, even across q-block boundaries
                # where ACT carries a 3-exp backlog.
                if kc >= 2:
                    ensure(("v", b * KC + kc - 2))
                    emit_attnv(b, qb, kc - 2, pts[kc - 2], acc0, acc1)
            for j in (KC - 2, KC - 1):
                ensure(("v", b * KC + j))
                emit_attnv(b, qb, j, pts[j], acc0, acc1)

            # ---- normalize ----
            # custom-DVE ops can't read PSUM on hardware: hop the denominator
            # rows through SBUF before the fast reciprocal. The acc data rows
            # also hop to SBUF immediately — this frees the acc PSUM banks for
            # the next q-block and satisfies the TensorTensor one-PSUM rule
            # (the muls then read sbuf-acc x psum-rep).
            rec_bf = small.tile([65, 1024], BF, tag="rec_bf", name=f"recb{b}_{qb}")
            a0_sb = small.tile([64, 512], F32, tag="a0sb", name=f"a0s{b}_{qb}")
            a1_sb = small.tile([64, 512], F32, tag="a1sb", name=f"a1s{b}_{qb}")
            if (b, qb) == last:
                # tail: den rows hop via the otherwise-idle ACT engine so the
                # reciprocal chain and the acc copies run in parallel, and the
                # reciprocal/cast split per head so bcast0 starts sooner.
                nc.scalar.activation(den_t[64:65, 0:512], acc0[64:65, :], Copy)
                nc.scalar.activation(den_t[64:65, 512:1024], acc1[64:65, :], Copy)
                nc.vector.reciprocal_approx_fast(rec_t[:, 0:512], den_t[:, 0:512])
                nc.vector.tensor_copy(rec_bf[64:65, 0:512], rec_t[64:65, 0:512])
                nc.vector.tensor_copy(a0_sb[:], acc0[0:64, :])
                nc.vector.reciprocal_approx_fast(rec_t[:, 512:1024], den_t[:, 512:1024])
                nc.vector.tensor_copy(rec_bf[64:65, 512:1024], rec_t[64:65, 512:1024])
                nc.vector.tensor_copy(a1_sb[:], acc1[0:64, :])
            else:
                nc.vector.tensor_copy(den_t[64:65, 0:512], acc0[64:65, :])
                nc.vector.tensor_copy(den_t[64:65, 512:1024], acc1[64:65, :])
                nc.vector.reciprocal_approx_fast(rec_t[:], den_t[:])
                nc.vector.tensor_copy(rec_bf[64:65, :], rec_t[64:65, :])
                nc.vector.tensor_copy(a0_sb[:], acc0[0:64, :])
                nc.vector.tensor_copy(a1_sb[:], acc1[0:64, :])

            def norm_post():
                # NOTE: gpsimd.partition_broadcast always reads partition 0
                # on hardware (AP partition offset ignored), so the recip
                # rows at partition 64 must broadcast via PE matmuls.
                rep0 = ps_work.tile([64, 512], F32, tag="work", name=f"rep0_{b}_{qb}")
                nc.tensor.matmul(
                    rep0[:], ones_t[64:65, :], rec_bf[64:65, 0:512],
                    start=True, stop=True, tile_position=(64, 0),
                )
                rep1 = ps_work.tile([64, 512], F32, tag="work", name=f"rep1_{b}_{qb}")
                nc.tensor.matmul(
                    rep1[:], ones_t[64:65, :], rec_bf[64:65, 512:1024],
                    start=True, stop=True, tile_position=(64, 0),
                )
                nc.vector.tensor_mul(
                    outT[0:64, bass.ts(qb, 512)], a0_sb[:], rep0[:]
                )
                tmp = small.tile([64, 512], BF, tag="tmp", name=f"tmp{b}_{qb}")
                nc.vector.tensor_mul(tmp[:], a1_sb[:], rep1[:])
                state["tmp"] = tmp
                if (b, qb) != last:
                    nc.sync.dma_start(outT[64:128, bass.ts(qb, 512)], tmp[:])
                if DEBUG_DUMPS and b == 0 and qb == 0:
                    for dn, dt_ in (
                        ("rec", rec_t[64:65, 0:512]),
                        ("rec1", rec_t[64:65, 512:1024]),
                        ("rep0", a0_sb[:]),
                        ("rep1", a1_sb[:]),
                        ("vaug0", vaug[:, 0:65]),
                        ("vaug1", vaug[:, 65:VW]),
                    ):
                        dbgt = nc.dram_tensor(
                            f"dbg_{dn}", list(dt_.shape), dt_.dtype, kind="ExternalOutput"
                        )
                        nc.sync.dma_start(dbgt[:], dt_)

            if DEBUG_DUMPS and qb == QB - 1:
                dbgo = nc.dram_tensor(f"dbg_outT{b}", [128, S], BF, kind="ExternalOutput")
                nc.sync.dma_start(dbgo[:], outT[:])
            if (b, qb) == last:
                # tail: held filler units execute on the PE while the DVE/ACT
                # normalize chain runs, then the split-K output projection.
                while filler or cur[1] is not None:
                    pump()
                norm_post()
                push_oproj(b, qb, outT, tail=True)
                while filler or cur[1] is not None:
                    pump()
            else:
                norm_post()
                push_oproj(b, qb, outT)


def kernel(x, mask, W_Q, W_K, W_V, W_O, _trace=False):
    # mask is all-ones for this problem; the reference `where(mask==0, -inf)` is a
    # no-op, so it is not shipped to the device.
    x = np.ascontiguousarray(np.asarray(x), dtype=np.float32)
    xT_bf = np.ascontiguousarray(np.asarray(x).reshape(BS, D).T).astype(BF16)

    if "nc" not in _nc_cache:
        _nc_cache["nc"] = _build_program()
    nc = _nc_cache["nc"]

    def prep_w(W, c):
        # device SBUF layout [p, cc*SLW + m] = W[sl].T[cc*128 + p, m]
        sl = slice(c * SLW, (c + 1) * SLW)
        wT = np.asarray(W)[sl, :].T.reshape(8, 128, SLW)  # [cc, p, m]
        return np.ascontiguousarray(wT.transpose(1, 0, 2).reshape(128, 8 * SLW)).astype(
            BF16
        )

    in_maps = []
    for c in range(N_CORES):
        sl = slice(c * SLW, (c + 1) * SLW)
        in_maps.append(
            {
                "xT": xT_bf,
                "wqT": prep_w(W_Q, c),
                "wkT": prep_w(W_K, c),
                "wvT": prep_w(W_V, c),
                "woT": np.ascontiguousarray(np.asarray(W_O)[:, sl].T).astype(BF16),
            }
        )

    res = run_bass_kernel_spmd(nc, in_maps, core_ids=list(range(N_CORES)), trace=_trace)
    _nc_cache["last_result"] = res

    total = np.zeros((D, BS), dtype=np.float32)
    for c in range(N_CORES):
        total += res.results[c]["out"].astype(np.float32)
    return np.ascontiguousarray(total.T).reshape(B, S, D)
